# revision 7
# baseline (speedup 1.0000x reference)
"""GNN NodeBlock (message passing + 3-layer MLP + LayerNorm) on 8 Trainium2 cores.

Strategy (data parallel over nodes):
  - Shard 50000 nodes across 8 cores (6250 each, padded to 6272 = 49*128).
  - Within each core, nodes are sorted by in-degree (ascending) so each
    128-node tile has a tight fixed edge capacity C_t (= max in-degree in
    that tile, maxed across cores so the program is SPMD-identical).
  - Edge features are laid out by the host in a transposed, per-node
    padded layout efT[f, (node p, slot c)] so the per-tile segment-sum is
    ONE VectorEngine tensor_reduce over the innermost axis:
        aggT[f, p] = sum_c efT[f, p, c]           (bf16 in, f32 out)
    This removes the aggregation entirely from the TensorEngine (which is
    the bottleneck: >94% busy on matmuls) at the cost of idle-DVE cycles.
    The f32 agg is cast to bf16 on the (otherwise idle) GpSimd engine.
  - The MLP runs in T-layout (features on partitions, nodes on the free
    dim) with weights stationary: h^T = W.T @ x^T, so no transposes are
    needed between layers. Node features enter pre-transposed (and
    degree-sort permuted) from the host.
  - Layer 3 swaps the operands (activations stationary) to produce y in
    natural layout [128 nodes, 512 feats]; b3 is added on the VectorEngine.
    LayerNorm reduces over the free dim: bn_stats/bn_aggr (VectorE) +
    sqrt (ScalarE) + reciprocal (VectorE), applied via one ScalarE
    activation with per-partition scale/bias.
  - All matmuls are bf16 inputs with fp32 PSUM accumulation.
  - The host undoes the degree-sort permutation on the output rows.

Everything is compiled once per (C-profile, apply_gamma_beta) and cached.
"""

import numpy as np
import ml_dtypes

P = 128
NODE_DIM = 512
EDGE_DIM = 96
HID = 1024
OUT = 512
N_NODES = 50000
N_EDGES = 800000
NCORES = 8
LN_EPS = 1e-5

NPC = N_NODES // NCORES          # 6250 nodes per core
T_TILES = -(-NPC // P)           # 49 node tiles per core
NPAD = T_TILES * P               # 6272
GMAX = 4                         # node tiles per super-tile (nt = 512 free dim)

BF16 = ml_dtypes.bfloat16

_CACHE: dict = {}


# ----------------------------------------------------------------------------
# Bass program
# ----------------------------------------------------------------------------

def _build_program(caps: tuple, apply_gamma_beta: bool):
    import concourse.bass as bass
    import concourse.bacc as bacc
    import concourse.mybir as mybir
    import concourse.tile as tile

    f32 = mybir.dt.float32
    bf16 = mybir.dt.bfloat16
    Act = mybir.ActivationFunctionType
    Alu = mybir.AluOpType
    AxL = mybir.AxisListType

    KD = NODE_DIM // P           # 4 node-feat k-chunks
    KH = HID // P                # 8 hidden k-chunks
    MH = HID // P                # 8 hidden m-chunks
    KD1 = KD + 1                 # + 1 chunk for the 96 agg features

    offs = np.zeros(T_TILES + 1, np.int64)
    np.cumsum(np.asarray(caps, np.int64) * P, out=offs[1:])
    tot = int(offs[-1])

    nc = bacc.Bacc("TRN2", target_bir_lowering=False, debug=False)

    # inputs (per core)
    efT_d = nc.declare_dram_parameter("efT", [EDGE_DIM, tot], bf16, isOutput=False)
    nfT_d = nc.declare_dram_parameter("nfT", [NODE_DIM, NPAD], bf16, isOutput=False)
    w1_d = nc.declare_dram_parameter("w1", [P, KD1 * MH * P], bf16, isOutput=False)
    w2_d = nc.declare_dram_parameter("w2", [P, KH * MH * P], bf16, isOutput=False)
    w3_d = nc.declare_dram_parameter("w3", [P, KH * OUT], bf16, isOutput=False)
    # cstB: b1T(MH) | b2T(MH); cstLN: gamma(OUT) | beta(OUT) | b3(OUT) | eps(1)
    cstB_d = nc.declare_dram_parameter("cstB", [P, 2 * MH], f32, isOutput=False)
    cstLN_d = nc.declare_dram_parameter("cstLN", [P, 3 * OUT + 1], f32, isOutput=False)
    y_d = nc.declare_dram_parameter("y", [NPAD, OUT], f32, isOutput=True)

    groups = []
    t0 = 0
    while t0 < T_TILES:
        g = min(GMAX, T_TILES - t0)
        groups.append((t0, g))
        t0 += g

    with tile.TileContext(nc) as tc:
        with (
            tc.tile_pool(name="const", bufs=1) as constp,
            tc.tile_pool(name="ef", bufs=4) as efp,
            tc.tile_pool(name="af", bufs=4) as afp,
            tc.tile_pool(name="agg", bufs=3) as aggp,
            tc.tile_pool(name="nfx", bufs=2) as nfxp,
            tc.tile_pool(name="h1", bufs=2) as h1p,
            tc.tile_pool(name="h2", bufs=2) as h2p,
            tc.tile_pool(name="yo", bufs=3) as yop,
            tc.tile_pool(name="st", bufs=8) as stp,
            tc.tile_pool(name="psM", bufs=4, space="PSUM") as psM,
            tc.tile_pool(name="psY", bufs=3, space="PSUM") as psY,
        ):
            # PE warm-up: garbage matmuls on a zeroed tile so the tensor
            # engine's DVFS ramp (slow p-state for the first ~3us of
            # activity) completes while the first real DMAs are in flight.
            warm = constp.tile([P, GMAX * P], bf16)
            nc.gpsimd.memset(warm[:], 0)
            ps_w = psM.tile([P, GMAX * P], f32, tag="psM")
            for i in range(14):
                nc.tensor.matmul(out=ps_w[:], lhsT=warm[:, 0:P],
                                 rhs=warm[:], start=(i == 0), stop=(i == 13))

            cstB_sb = constp.tile([P, 2 * MH], f32)
            w1_sb = constp.tile([P, MH * KD1 * P], bf16)
            w2_sb = constp.tile([P, MH * KH * P], bf16)
            w3_sb = constp.tile([P, KH * OUT], bf16)
            cstLN_sb = constp.tile([P, 3 * OUT + 1], f32)

            nfT_ap = nfT_d[:, :].rearrange("(k p) n -> p k n", p=P)

            # prologue DMA order = DMA-queue order: the first layer-1 chain
            # needs nfx + w1[m=0] + (agg tile 0..3); everything else after.
            nfx0 = nfxp.tile([P, KD, GMAX * P], bf16, tag="nfx")
            g0 = groups[0][1]
            nc.sync.dma_start(out=nfx0[:, :, 0:g0 * P], in_=nfT_ap[:, :, 0:g0 * P])
            nc.sync.dma_start(out=w1_sb[:, 0:KD1 * P], in_=w1_d[:, 0:KD1 * P])

            def emit_agg_subtile(t, aggT, s):
                """Segment-sum of tile t into aggT[:, s*P:(s+1)*P] via one
                DVE reduce over the per-node edge slots (+ GpSimd cast)."""
                c = caps[t]
                ef_t = efp.tile([EDGE_DIM, P, c], bf16, tag="ef", name="ef_t")
                nc.sync.dma_start(
                    out=ef_t[:],
                    in_=efT_d[:, offs[t]:offs[t + 1]].rearrange(
                        "f (p c) -> f p c", p=P),
                )
                af_t = afp.tile([EDGE_DIM, P], f32, tag="af", name="af_t")
                nc.vector.tensor_reduce(
                    out=af_t[:], in_=ef_t[:], axis=AxL.X, op=Alu.add)
                nc.gpsimd.tensor_copy(
                    out=aggT[:, s * P:(s + 1) * P], in_=af_t[:])

            # group 0's aggregation up front (weight DMAs stream in behind it)
            agg_tiles = {}
            agg_tiles[0] = aggp.tile([EDGE_DIM, GMAX * P], bf16, tag="agg",
                                     name="aggT")
            for s in range(groups[0][1]):
                emit_agg_subtile(groups[0][0] + s, agg_tiles[0], s)
            nc.sync.dma_start(out=cstB_sb[:], in_=cstB_d[:, :])
            for m in range(1, MH):
                nc.sync.dma_start(
                    out=w1_sb[:, m * KD1 * P:(m + 1) * KD1 * P],
                    in_=w1_d[:, m * KD1 * P:(m + 1) * KD1 * P])

            nfx_tiles = {0: nfx0}
            for gi, (tstart, g) in enumerate(groups):
                nt = g * P  # free-dim width of this super-tile
                n0 = tstart * P
                aggT = agg_tiles.pop(gi)
                nfx = nfx_tiles.pop(gi)

                # ---- layer 1: h1T[m] = relu(W1.T @ xT + b1), x = [nf; agg] ----
                h1 = h1p.tile([P, KH, GMAX * P], bf16, tag="h1")
                for m in range(MH):
                    ps = psM.tile([P, GMAX * P], f32, tag="psM")
                    for k in range(KD):
                        nc.tensor.matmul(
                            out=ps[:, 0:nt],
                            lhsT=w1_sb[:, (m * KD1 + k) * P:(m * KD1 + k + 1) * P],
                            rhs=nfx[:, k, 0:nt],
                            start=(k == 0),
                            stop=False,
                        )
                    if gi == 0:
                        # per-subtile agg matmuls: each waits only on its own
                        # subtile's reduce+cast, so layer 1 starts while the
                        # later agg subtiles are still streaming in.
                        for s in range(g):
                            nc.tensor.matmul(
                                out=ps[:, s * P:(s + 1) * P],
                                lhsT=w1_sb[0:EDGE_DIM,
                                           (m * KD1 + KD) * P:(m * KD1 + KD) * P + P],
                                rhs=aggT[:, s * P:(s + 1) * P],
                                start=False,
                                stop=True,
                            )
                    else:
                        nc.tensor.matmul(
                            out=ps[:, 0:nt],
                            lhsT=w1_sb[0:EDGE_DIM, (m * KD1 + KD) * P:(m * KD1 + KD) * P + P],
                            rhs=aggT[:, 0:nt],
                            start=False,
                            stop=True,
                        )
                    nc.scalar.activation(
                        out=h1[:, m, 0:nt], in_=ps[:, 0:nt], func=Act.Relu,
                        bias=cstB_sb[:, m:m + 1],
                    )
                    if gi == 0:
                        nc.sync.dma_start(
                            out=w2_sb[:, m * KH * P:(m + 1) * KH * P],
                            in_=w2_d[:, m * KH * P:(m + 1) * KH * P])

                # ---- layer 2 ----
                h2 = h2p.tile([P, KH, GMAX * P], bf16, tag="h2")
                for m in range(MH):
                    ps = psM.tile([P, GMAX * P], f32, tag="psM")
                    for k in range(KH):
                        nc.tensor.matmul(
                            out=ps[:, 0:nt],
                            lhsT=w2_sb[:, (m * KH + k) * P:(m * KH + k + 1) * P],
                            rhs=h1[:, k, 0:nt],
                            start=(k == 0),
                            stop=(k == KH - 1),
                        )
                    nc.scalar.activation(
                        out=h2[:, m, 0:nt], in_=ps[:, 0:nt], func=Act.Relu,
                        bias=cstB_sb[:, MH + m:MH + m + 1],
                    )
                    if gi == 0 and m < 2:
                        if m == 0:
                            nc.sync.dma_start(out=w3_sb[:], in_=w3_d[:, :])
                        else:
                            nc.sync.dma_start(out=cstLN_sb[:], in_=cstLN_d[:, :])

                # ---- layer 3 (nodes on partitions) + LayerNorm ----
                # aggregation (DVE) for group gi+1 interleaves here so its
                # results are ready before the next group's layer-1 chain.
                if gi + 1 < len(groups):
                    tstart_nx, g_nx = groups[gi + 1]
                    agg_tiles[gi + 1] = aggp.tile([EDGE_DIM, GMAX * P], bf16,
                                                  tag="agg", name="aggT")
                    # prefetch next group's node features ahead of its edges
                    nfx_nx = nfxp.tile([P, KD, GMAX * P], bf16, tag="nfx")
                    nc.sync.dma_start(
                        out=nfx_nx[:, :, 0:g_nx * P],
                        in_=nfT_ap[:, :, tstart_nx * P:(tstart_nx + g_nx) * P])
                    nfx_tiles[gi + 1] = nfx_nx
                else:
                    tstart_nx, g_nx = 0, 0
                # all of next group's reduces go on the DVE queue before this
                # group's LN chains: the in-order DVE would otherwise sit in
                # a b3-add stall with the reduces stuck behind it.
                for s in range(g_nx):
                    emit_agg_subtile(tstart_nx + s, agg_tiles[gi + 1], s)
                for s in range(g):
                    ps_y = psY.tile([P, OUT], f32, tag="psY")
                    for k in range(KH):
                        nc.tensor.matmul(
                            out=ps_y[:],
                            lhsT=h2[:, k, s * P:(s + 1) * P],
                            rhs=w3_sb[:, k * OUT:(k + 1) * OUT],
                            start=(k == 0),
                            stop=(k == KH - 1),
                        )
                    # + b3 (broadcast rows) on VectorE, off the TensorE critical path
                    nc.vector.tensor_tensor(
                        out=ps_y[:], in0=ps_y[:],
                        in1=cstLN_sb[:, 2 * OUT:3 * OUT],
                        op=Alu.add,
                    )
                    st6 = stp.tile([P, 6], f32, tag="st6")
                    nc.vector.bn_stats(st6[:], ps_y[:])
                    mv = stp.tile([P, 2], f32, tag="mv")
                    nc.vector.bn_aggr(mv[:], st6[:])
                    std = stp.tile([P, 1], f32, tag="std")
                    nc.scalar.activation(std[:], mv[:, 1:2], Act.Sqrt,
                                         bias=cstLN_sb[:, 3 * OUT:])
                    rstd = stp.tile([P, 1], f32, tag="rstd")
                    nc.vector.reciprocal(rstd[:], std[:])
                    nmr = stp.tile([P, 1], f32, tag="nmr")
                    nc.vector.tensor_scalar(
                        out=nmr[:], in0=mv[:, 0:1], scalar1=rstd[:], scalar2=-1.0,
                        op0=Alu.mult, op1=Alu.mult,
                    )
                    yn = yop.tile([P, OUT], f32, tag="yn")
                    nc.scalar.activation(
                        out=yn[:], in_=ps_y[:], func=Act.Identity,
                        bias=nmr[:], scale=rstd[:],
                    )
                    if apply_gamma_beta:
                        nc.vector.tensor_tensor(
                            out=yn[:], in0=yn[:],
                            in1=cstLN_sb[:, 0:OUT], op=Alu.mult,
                        )
                        nc.vector.tensor_tensor(
                            out=yn[:], in0=yn[:],
                            in1=cstLN_sb[:, OUT:2 * OUT], op=Alu.add,
                        )
                    r0 = (tstart + s) * P
                    nc.sync.dma_start(out=y_d[r0:r0 + P, :], in_=yn[:])

    nc.compile()
    return nc


# ----------------------------------------------------------------------------
# Host-side sharding / layout prep
# ----------------------------------------------------------------------------

def _order_core(c, dst):
    """Degree-ascending node order for core c; returns (order, deg_sorted)."""
    lo = c * NPC
    sel = np.flatnonzero((dst >= lo) & (dst < lo + NPC))
    d = (dst[sel] - lo).astype(np.int64)
    deg = np.bincount(d, minlength=NPC)
    order = np.argsort(deg, kind="stable")
    return sel, d, deg, order


def _prep_core(sel, d, deg, order, caps, offs, node_feat, edge_feat, lo):
    rank = np.empty(NPC, np.int64)
    rank[order] = np.arange(NPC)
    r = rank[d]
    # within-node slot index (edges of the same node get consecutive slots)
    sort_by_r = np.argsort(r, kind="stable")
    r_sorted = r[sort_by_r]
    sel_sorted = sel[sort_by_r]
    node_starts = np.zeros(NPC, np.int64)
    cnt = np.bincount(r_sorted, minlength=NPC)
    np.cumsum(cnt[:-1], out=node_starts[1:])
    slot = np.arange(r_sorted.size) - node_starts[r_sorted]

    t = r_sorted >> 7
    p = r_sorted & 127
    cap_t = caps[t]
    col = offs[t] + p * cap_t + slot

    tot = int(offs[-1])
    efT = np.zeros((EDGE_DIM, tot), BF16)
    efT[:, col] = edge_feat[sel_sorted].T.astype(BF16)

    nfT = np.zeros((NODE_DIM, NPAD), BF16)
    nfT[:, :NPC] = node_feat[lo:lo + NPC][order].T.astype(BF16)
    return {"efT": efT, "nfT": nfT}


def _prep_shared(W1, b1, W2, b2, W3, b3, gamma, beta):
    KD1 = NODE_DIM // P + 1
    MH = HID // P
    KH = HID // P

    w1p = np.zeros((KD1 * P, HID), np.float32)
    w1p[:NODE_DIM + EDGE_DIM] = W1
    # m-major: col index (m*KD1 + k)*P + j
    w1 = np.ascontiguousarray(
        w1p.reshape(KD1, P, MH, P).transpose(1, 2, 0, 3)).reshape(P, -1).astype(BF16)
    w2 = np.ascontiguousarray(
        W2.reshape(KH, P, MH, P).transpose(1, 2, 0, 3)).reshape(P, -1).astype(BF16)
    w3 = np.ascontiguousarray(
        W3.reshape(KH, P, OUT).transpose(1, 0, 2)).reshape(P, -1).astype(BF16)

    cstB = np.ascontiguousarray(np.concatenate(
        [b1.reshape(MH, P).T, b2.reshape(MH, P).T], axis=1).astype(np.float32))
    cstLN = np.ascontiguousarray(np.concatenate([
        np.tile(gamma.reshape(1, OUT), (P, 1)),
        np.tile(beta.reshape(1, OUT), (P, 1)),
        np.tile(b3.reshape(1, OUT), (P, 1)),
        np.full((P, 1), LN_EPS, np.float32),
    ], axis=1).astype(np.float32))
    return {"w1": w1, "w2": w2, "w3": w3, "cstB": cstB, "cstLN": cstLN}


# ----------------------------------------------------------------------------
# Entry point
# ----------------------------------------------------------------------------

def _ensure_axon_hooks_importable():
    """bass_utils imports antenv.axon_hooks when tracing is requested (even via
    the BASS_TRACE env var); provide a no-op stub if the module is absent so
    that path degrades to trace-skipped instead of crashing."""
    try:
        import antenv.axon_hooks  # noqa: F401
    except Exception:
        import sys
        import types
        try:
            import antenv
        except Exception:
            return
        mod = types.ModuleType('antenv.axon_hooks')
        mod._hook = None
        mod.set_axon_ntff_profile_hook = lambda h: setattr(mod, '_hook', h)
        mod.get_axon_ntff_profile_hook = lambda: mod._hook
        sys.modules['antenv.axon_hooks'] = mod
        antenv.axon_hooks = mod


def kernel(node_feat, edge_feat, edge_index, n_nodes, W1, b1, W2, b2, W3, b3,
           gamma, beta, _want_trace=False):
    from concourse.bass_utils import run_bass_kernel_spmd
    _ensure_axon_hooks_importable()

    node_feat = np.asarray(node_feat, dtype=np.float32)
    edge_feat = np.asarray(edge_feat, dtype=np.float32)
    edge_index = np.asarray(edge_index)
    assert int(n_nodes) == N_NODES
    assert node_feat.shape == (N_NODES, NODE_DIM)
    assert edge_feat.shape == (N_EDGES, EDGE_DIM)

    dst = edge_index[1].astype(np.int64)

    cores = [_order_core(c, dst) for c in range(NCORES)]
    # per-tile capacity = max in-degree in tile, maxed across cores (SPMD)
    caps = np.ones(T_TILES, np.int64)
    for sel, d, deg, order in cores:
        degs = np.zeros(NPAD, np.int64)
        degs[:NPC] = deg[order]
        caps = np.maximum(caps, degs.reshape(T_TILES, P).max(axis=1))
    offs = np.zeros(T_TILES + 1, np.int64)
    np.cumsum(caps * P, out=offs[1:])

    gamma = np.asarray(gamma, dtype=np.float32)
    beta = np.asarray(beta, dtype=np.float32)
    apply_gb = not (np.all(gamma == 1.0) and np.all(beta == 0.0))

    key = (tuple(caps.tolist()), apply_gb)
    if key not in _CACHE:
        _CACHE[key] = _build_program(key[0], apply_gb)
    nc = _CACHE[key]

    shared = _prep_shared(
        np.asarray(W1, np.float32), np.asarray(b1, np.float32),
        np.asarray(W2, np.float32), np.asarray(b2, np.float32),
        np.asarray(W3, np.float32), np.asarray(b3, np.float32),
        gamma, beta)

    in_maps = []
    for c in range(NCORES):
        sel, d, deg, order = cores[c]
        m = _prep_core(sel, d, deg, order, caps, offs, node_feat, edge_feat,
                       c * NPC)
        m.update(shared)
        in_maps.append(m)

    res = run_bass_kernel_spmd(nc, in_maps, list(range(NCORES)), trace=_want_trace)

    out = np.empty((N_NODES, OUT), np.float32)
    for c in range(NCORES):
        order = cores[c][3]
        yc = res.results[c]["y"][:NPC]
        out[c * NPC + order] = yc
    if _want_trace:
        kernel.last_results = res
    return out


kernel.last_results = None
